# revision 1
# baseline (speedup 1.0000x reference)
"""Cross-attention Trainium2 kernel (8 NeuronCores, SPMD).

Sharding: core = 2*b + hh  (b = batch 0..3, hh = head-half 0..1).
Each core computes attention for one batch and 8 of the 16 heads, plus the
partial output projection for its head block; the host sums the two partial
projections per batch.

Per-core dataflow (all on-chip after the initial loads):
  - kT[hd, s], qT[hd, t] head-transposed projections via PE matmuls
    (host supplies src/tgt pre-transposed and pre-tiled so every DMA is
    16KB-contiguous per partition)
  - scoresT[s, t] = kT.T-slice @ qT  (two heads packed via PE row tiling)
  - p = exp(scores/8 + mask_bias)    (mask enters as the per-partition
    activation bias: s is the partition axis of scoresT)
  - attn@V with stationary [v | ones]: row 64 of the PSUM accumulator is the
    softmax denominator for free
  - normalize via fast reciprocal + GpSimd partition-broadcast + DVE multiply
  - partial out-projection (delayed one chunk to hide the normalization
    latency), PSUM bounced through SBUF to DRAM

All matmuls run in float32r mode (fp32 data, full-rate PE).
"""

import os
import sys

import numpy as np


def _ensure_paths():
    for p in ("/opt/trn_rl_repo", "/root/.axon_site/_ro/trn_rl_repo"):
        if os.path.isdir(p) and p not in sys.path:
            sys.path.insert(0, p)


_ensure_paths()

import concourse.bass as bass  # noqa: E402
import concourse.mybir as mybir  # noqa: E402
from concourse import bacc  # noqa: E402
from concourse.bass_utils import run_bass_kernel_spmd  # noqa: E402
from concourse.tile import TileContext  # noqa: E402

B, S, T, D, H = 4, 2048, 2048, 1024, 16
HD = D // H  # 64
HL = H // 2  # heads per core: 8
HDL = HL * HD  # 512 head dims per core
PAIRS = HL // 2  # 4 head pairs (2 heads share a 128-partition tile)
KT = D // 128  # 8 contraction k-tiles for the projections
CH = 4  # t-chunks of 512
CHW = 512
STN = S // 128  # 16 source tiles
F32 = mybir.dt.float32
F32R = mybir.dt.float32r

N_CORES = 8

_PROG = None
_last_in_maps = None


def _build_program():
    nc = bacc.Bacc(None, target_bir_lowering=False, debug=False)

    # Host-pre-tiled inputs: every DMA lands 16KB-contiguous per partition.
    tgtS = nc.dram_tensor("tgtS", [CH, 128, KT, CHW], F32R, kind="ExternalInput")
    srcS = nc.dram_tensor("srcS", [CH, 128, KT, CHW], F32R, kind="ExternalInput")
    wqS = nc.dram_tensor("wqS", [128, KT, HDL], F32R, kind="ExternalInput")
    wkS = nc.dram_tensor("wkS", [128, KT, HDL], F32R, kind="ExternalInput")
    wvS = nc.dram_tensor("wvS", [128, KT, HDL], F32R, kind="ExternalInput")
    woW = nc.dram_tensor("woW", [128, PAIRS, D], F32R, kind="ExternalInput")
    mbias = nc.dram_tensor("mbias", [128, STN], F32, kind="ExternalInput")
    ones_in = nc.dram_tensor("ones_in", [128, 128], F32R, kind="ExternalInput")
    outp = nc.dram_tensor("outp", [T, D], F32, kind="ExternalOutput")

    Exp = mybir.ActivationFunctionType.Exp

    with nc.allow_low_precision("fp32r matmul inputs"), TileContext(nc) as tc:
        with (
            tc.tile_pool(name="const", bufs=1) as const_pool,
            tc.tile_pool(name="w", bufs=1) as w_pool,
            tc.tile_pool(name="kv", bufs=1) as kv_pool,
            tc.tile_pool(name="stream", bufs=2) as stream_pool,
            tc.tile_pool(name="qc", bufs=2) as qc_pool,
            tc.tile_pool(name="pt", bufs=2) as pt_pool,
            tc.tile_pool(name="on", bufs=2) as on_pool,
            tc.tile_pool(name="osb", bufs=2) as osb_pool,
            tc.tile_pool(name="rcb", bufs=2) as rcb_pool,
            tc.tile_pool(name="avs", bufs=2) as avs_pool,
            tc.tile_pool(name="acc_ps", bufs=2, space="PSUM") as acc_ps,
            tc.tile_pool(name="av_ps", bufs=1, space="PSUM") as av_ps_pool,
            tc.tile_pool(name="sc_ps", bufs=2, space="PSUM") as sc_ps_pool,
        ):
            # constants / weights (all loaded up front; no pool churn)
            MB = const_pool.tile([128, STN], F32)
            nc.sync.dma_start(out=MB, in_=mbias[:, :])
            WK = w_pool.tile([128, KT, HDL], F32R, tag="wk")
            nc.sync.dma_start(out=WK, in_=wkS[:, :, :])
            WV = w_pool.tile([128, KT, HDL], F32R, tag="wv")
            nc.sync.dma_start(out=WV, in_=wvS[:, :, :])
            WQ = w_pool.tile([128, KT, HDL], F32R, tag="wq")
            nc.sync.dma_start(out=WQ, in_=wqS[:, :, :])
            # persistent K/V for the attention phase
            KTt = kv_pool.tile([128, PAIRS, S], F32R)
            VON = kv_pool.tile([128, STN, HL * (HD + 1)], F32R)
            von_heads = VON[:, :, :].rearrange("p s (h e) -> p s h e", e=HD + 1)
            nc.sync.dma_start(
                out=von_heads[:, :, :, HD],
                in_=ones_in[:, 0:128].rearrange("p (s h) -> p s h", s=STN),
            )

            # ---- source-side projections: kT and v ----
            for c in range(CH):
                SRC = stream_pool.tile([128, KT, CHW], F32R, tag="stream", name="SRC")
                nc.sync.dma_start(out=SRC, in_=srcS[c, :, :, :])
                for j in range(PAIRS):
                    k_ps = acc_ps.tile([128, CHW], F32, tag="acc", name="k_ps")
                    for k in range(KT):
                        nc.tensor.matmul(
                            k_ps,
                            lhsT=WK[:, k, j * 128 : (j + 1) * 128],
                            rhs=SRC[:, k, :],
                            start=(k == 0),
                            stop=(k == KT - 1),
                        )
                    nc.vector.tensor_copy(KTt[:, j, c * CHW : (c + 1) * CHW], k_ps)
                for stl in range(4):
                    st = c * 4 + stl
                    v_ps = acc_ps.tile([128, CHW], F32, tag="acc", name="v_ps")
                    for k in range(KT):
                        nc.tensor.matmul(
                            v_ps[:, 0:HDL],
                            lhsT=SRC[:, k, stl * 128 : (stl + 1) * 128],
                            rhs=WV[:, k, :],
                            start=(k == 0),
                            stop=(k == KT - 1),
                        )
                    nc.vector.tensor_copy(
                        von_heads[:, st, :, 0:HD],
                        v_ps[:, 0:HDL].rearrange("p (h e) -> p h e", e=HD),
                    )

            # WO reuses WV's slot (tag) — WV is dead once the src phase ends
            WO = w_pool.tile([128, PAIRS, D], F32R, tag="wv", name="WO")
            nc.sync.dma_start(out=WO, in_=woW[:, :, :])

            # ---- target projections + attention (+ out-proj delayed 1 chunk) --
            OTN_tiles = {}

            def emit_outproj(cc):
                OTNp = OTN_tiles.pop(cc)
                for ttl in range(4):
                    for dc in range(2):
                        o_ps = acc_ps.tile([128, CHW], F32, tag="acc", name="o_ps")
                        for j in range(PAIRS):
                            nc.tensor.matmul(
                                o_ps,
                                lhsT=OTNp[:, j, ttl * 128 : (ttl + 1) * 128],
                                rhs=WO[:, j, dc * CHW : (dc + 1) * CHW],
                                start=(j == 0),
                                stop=(j == PAIRS - 1),
                            )
                        OSB = osb_pool.tile([128, CHW], F32, tag="osb", name="OSB")
                        nc.vector.tensor_copy(OSB, o_ps)
                        row0 = cc * CHW + ttl * 128
                        nc.sync.dma_start(
                            out=outp[row0 : row0 + 128, dc * CHW : (dc + 1) * CHW],
                            in_=OSB,
                        )

            for c in range(CH):
                TGT = stream_pool.tile([128, KT, CHW], F32R, tag="stream", name="TGT")
                nc.sync.dma_start(out=TGT, in_=tgtS[c, :, :, :])
                QTc = qc_pool.tile([128, PAIRS, CHW], F32R)
                for j in range(PAIRS):
                    q_ps = acc_ps.tile([128, CHW], F32, tag="acc", name="q_ps")
                    for k in range(KT):
                        nc.tensor.matmul(
                            q_ps,
                            lhsT=WQ[:, k, j * 128 : (j + 1) * 128],
                            rhs=TGT[:, k, :],
                            start=(k == 0),
                            stop=(k == KT - 1),
                        )
                    nc.vector.tensor_copy(QTc[:, j, :], q_ps)

                OTN = on_pool.tile([128, PAIRS, CHW], F32R)
                OTN_tiles[c] = OTN
                for j in range(PAIRS):
                    av = av_ps_pool.tile([128, 2 * CHW], F32, tag="av", name="av")
                    for st in range(STN):
                        sc = sc_ps_pool.tile([128, 2 * CHW], F32, tag="sc", name="sc")
                        nc.tensor.matmul(
                            sc[:, 0:CHW],
                            lhsT=KTt[0:64, j, st * 128 : (st + 1) * 128],
                            rhs=QTc[0:64, j, :],
                            start=True,
                            stop=True,
                        )
                        nc.tensor.matmul(
                            sc[:, CHW : 2 * CHW],
                            lhsT=KTt[64:128, j, st * 128 : (st + 1) * 128],
                            rhs=QTc[64:128, j, :],
                            start=True,
                            stop=True,
                        )
                        PT = pt_pool.tile([128, 2 * CHW], F32R)
                        nc.scalar.activation(
                            PT, sc, Exp, bias=MB[:, st : st + 1], scale=1.0 / 8.0
                        )
                        nc.tensor.matmul(
                            av[0:65, 0:CHW],
                            lhsT=VON[:, st, j * 130 : j * 130 + 65],
                            rhs=PT[:, 0:CHW],
                            start=(st == 0),
                            stop=(st == STN - 1),
                        )
                        nc.tensor.matmul(
                            av[0:65, CHW : 2 * CHW],
                            lhsT=VON[:, st, j * 130 + 65 : j * 130 + 130],
                            rhs=PT[:, CHW : 2 * CHW],
                            start=(st == 0),
                            stop=(st == STN - 1),
                        )
                    AVS = avs_pool.tile([128, 2 * CHW], F32, tag="avs", name="AVS")
                    nc.vector.tensor_copy(AVS[0:65, :], av[0:65, :])
                    RCL = rcb_pool.tile([1, 2 * CHW], F32, tag="rcb", name="RCL")
                    nc.scalar.activation(
                        RCL, AVS[64:65, :], mybir.ActivationFunctionType.Ln
                    )
                    RC = rcb_pool.tile([1, 2 * CHW], F32, tag="rcb", name="RC")
                    nc.scalar.activation(
                        RC, RCL, mybir.ActivationFunctionType.Exp, scale=-1.0
                    )
                    BCS = rcb_pool.tile([64, 2 * CHW], F32, tag="rcb", name="BCS")
                    nc.gpsimd.partition_broadcast(BCS, RC[0:1, :])
                    nc.vector.tensor_mul(
                        OTN[0:64, j, :], AVS[0:64, 0:CHW], BCS[:, 0:CHW]
                    )
                    STG = osb_pool.tile([128, CHW], F32R, tag="osb", name="STG")
                    nc.vector.tensor_mul(
                        STG[0:64, :], AVS[0:64, CHW : 2 * CHW], BCS[:, CHW : 2 * CHW]
                    )
                    nc.sync.dma_start(out=OTN[64:128, j, :], in_=STG[0:64, :])

                    if c > 0 and j == 0:
                        emit_outproj(c - 1)

            emit_outproj(CH - 1)

    nc.finalize()
    return nc


def _get_program():
    global _PROG
    if _PROG is None:
        _PROG = _build_program()
    return _PROG


def kernel(src, tgt, attention_mask, Wq, Wk, Wv, Wo):
    src = np.asarray(src, dtype=np.float32)
    tgt = np.asarray(tgt, dtype=np.float32)
    mask = np.asarray(attention_mask)
    Wq = np.asarray(Wq, dtype=np.float32)
    Wk = np.asarray(Wk, dtype=np.float32)
    Wv = np.asarray(Wv, dtype=np.float32)
    Wo = np.asarray(Wo, dtype=np.float32)

    nc = _get_program()

    ones_arr = np.ones((128, 128), dtype=np.float32)
    in_maps = []
    for core in range(N_CORES):
        b, hh = core // 2, core % 2
        rows = slice(hh * HDL, (hh + 1) * HDL)
        mb = np.where(mask[b], 0.0, -30000.0).astype(np.float32)
        # [T, D] -> [CH, 128p, KT, CHW]: x[c, p, k, t] = ten[c*CHW + t, k*128 + p]
        tgtSa = np.ascontiguousarray(
            tgt[b].reshape(CH, CHW, KT, 128).transpose(0, 3, 2, 1)
        )
        srcSa = np.ascontiguousarray(
            src[b].reshape(CH, CHW, KT, 128).transpose(0, 3, 2, 1)
        )
        # W[hd_local, d] -> [128p, KT, HDL]: w[p, k, h] = W[rows][h, k*128 + p]
        wqSa = np.ascontiguousarray(Wq[rows].reshape(HDL, KT, 128).transpose(2, 1, 0))
        wkSa = np.ascontiguousarray(Wk[rows].reshape(HDL, KT, 128).transpose(2, 1, 0))
        wvSa = np.ascontiguousarray(Wv[rows].reshape(HDL, KT, 128).transpose(2, 1, 0))
        # Wo[:, cols] -> [128p, PAIRS, D]: wo[p, j, d] = Wo[d, hh*HDL + j*128 + p]
        woWa = np.ascontiguousarray(
            Wo[:, rows].T.reshape(PAIRS, 128, D).transpose(1, 0, 2)
        )
        in_maps.append(
            {
                "tgtS": tgtSa,
                "srcS": srcSa,
                "wqS": wqSa,
                "wkS": wkSa,
                "wvS": wvSa,
                "woW": woWa,
                "mbias": np.ascontiguousarray(mb.reshape(STN, 128).T),
                "ones_in": ones_arr,
            }
        )

    global _last_in_maps
    _last_in_maps = in_maps

    res = run_bass_kernel_spmd(nc, in_maps, list(range(N_CORES)))

    out = np.empty((B, T, D), dtype=np.float32)
    for b in range(B):
        out[b] = res.results[2 * b]["outp"] + res.results[2 * b + 1]["outp"]
    return out



# revision 5
# speedup vs baseline: 1.4264x; 1.4264x over previous
"""Cross-attention Trainium2 kernel (8 NeuronCores, SPMD), v2.

Sharding: core = 2*b + hh  (b = batch 0..3, hh = head-half 0..1).
Each core computes attention for one batch and 8 of the 16 heads, plus the
partial output projection for its head block; the host sums the two partial
projections per batch.

v2 changes vs v1 (609us baseline):
  - fp16 matmul inputs everywhere: fp32r moving-data streams at half rate on
    TRN2 silicon (measured ~500ns for N=512 even at HAM K=8/8), fp16 streams
    full rate and enables Fast Weight Load. Scores stay fp32 in PSUM; exp
    reads fp32, writes fp16 P tiles.
  - host-side mask compaction: only the valid source rows are shipped and
    projected (dynamic ST_N source tiles, program compiled per ST_N). The
    mask bias disappears entirely: padded rows have K=0 -> exp(0)=1, and the
    VON "ones" column is the valid-mask so pad rows contribute neither to
    attn@V nor to the softmax denominator. 1/sqrt(HD) is folded into Wq on
    the host.
  - softmax reciprocal via vector.reciprocal_approx_fast (single DVE op)
    instead of ACT Ln+Exp: the scalar engine now only ever runs Exp, so
    exactly one ACT table load instead of 33.
  - software-pipelined attention inner loop: attn@V matmuls run one st-tile
    behind the scores so the PE never sits directly behind the exp.
  - out-projection and next-chunk Q-projection are emitted as small work
    items spread across the (ACT-bound) attention iterations.
"""

import os
import sys

import numpy as np


def _ensure_paths():
    for p in ("/opt/trn_rl_repo", "/root/.axon_site/_ro/trn_rl_repo"):
        if os.path.isdir(p) and p not in sys.path:
            sys.path.insert(0, p)


_ensure_paths()

import concourse.bass as bass  # noqa: E402
import concourse.mybir as mybir  # noqa: E402
from concourse import bacc  # noqa: E402
from concourse.bass_utils import run_bass_kernel_spmd  # noqa: E402
from concourse.tile import TileContext  # noqa: E402

B, S, T, D, H = 4, 2048, 2048, 1024, 16
HD = D // H  # 64
HL = H // 2  # heads per core: 8
HDL = HL * HD  # 512 head dims per core
PAIRS = HL // 2  # 4 head pairs (2 heads share a 128-partition tile)
KT = D // 128  # 8 contraction k-tiles for the projections
CH = 4  # t-chunks of 512
CHW = 512
F32 = mybir.dt.float32
F16 = mybir.dt.float16

N_CORES = 8

_PROGS = {}
_LAST_ST_N = None
_last_in_maps = None


def _build_program(st_n):
    sp = st_n * 128  # padded (compacted) source length
    scn = (sp + CHW - 1) // CHW  # source stream chunks of 512
    spad = scn * CHW

    nc = bacc.Bacc(None, target_bir_lowering=False, debug=False)

    # Host-pre-tiled inputs: every DMA lands contiguous per partition.
    tgtS = nc.dram_tensor("tgtS", [CH, 128, KT, CHW], F16, kind="ExternalInput")
    srcS = nc.dram_tensor("srcS", [scn, 128, KT, CHW], F16, kind="ExternalInput")
    wqS = nc.dram_tensor("wqS", [128, KT, HDL], F16, kind="ExternalInput")
    wkS = nc.dram_tensor("wkS", [128, KT, HDL], F16, kind="ExternalInput")
    wvS = nc.dram_tensor("wvS", [128, KT, HDL], F16, kind="ExternalInput")
    woW = nc.dram_tensor("woW", [128, PAIRS, D], F16, kind="ExternalInput")
    validS = nc.dram_tensor("validS", [128, st_n, HL], F16, kind="ExternalInput")
    outp = nc.dram_tensor("outp", [T, D], F32, kind="ExternalOutput")

    Exp = mybir.ActivationFunctionType.Exp

    with nc.allow_low_precision("fp16 matmul inputs"), TileContext(nc) as tc:
        with (
            tc.tile_pool(name="w", bufs=1) as w_pool,
            tc.tile_pool(name="kv", bufs=1) as kv_pool,
            tc.tile_pool(name="stream", bufs=2) as stream_pool,
            tc.tile_pool(name="qc", bufs=2) as qc_pool,
            tc.tile_pool(name="pt", bufs=2) as pt_pool,
            tc.tile_pool(name="on", bufs=2) as on_pool,
            tc.tile_pool(name="osb", bufs=2) as osb_pool,
            tc.tile_pool(name="rcb", bufs=2) as rcb_pool,
            tc.tile_pool(name="avs", bufs=2) as avs_pool,
            tc.tile_pool(name="acc_ps", bufs=2, space="PSUM") as acc_ps,
            tc.tile_pool(name="av_ps", bufs=1, space="PSUM") as av_ps_pool,
            tc.tile_pool(name="sc_ps", bufs=2, space="PSUM") as sc_ps_pool,
        ):
            WK = w_pool.tile([128, KT, HDL], F16, tag="wk")
            nc.sync.dma_start(out=WK, in_=wkS[:, :, :])
            WV = w_pool.tile([128, KT, HDL], F16, tag="wv")
            nc.sync.dma_start(out=WV, in_=wvS[:, :, :])
            WQ = w_pool.tile([128, KT, HDL], F16, tag="wq")
            nc.sync.dma_start(out=WQ, in_=wqS[:, :, :])
            # persistent K/V for the attention phase
            KTt = kv_pool.tile([128, PAIRS, spad], F16)
            VON = kv_pool.tile([128, st_n, HL * (HD + 1)], F16)
            von_heads = VON[:, :, :].rearrange("p s (h e) -> p s h e", e=HD + 1)
            nc.sync.dma_start(out=von_heads[:, :, :, HD], in_=validS[:, :, :])

            # ---- source-side projections: kT and v ----
            for c in range(scn):
                SRC = stream_pool.tile([128, KT, CHW], F16, tag="stream", name="SRC")
                nc.sync.dma_start(out=SRC, in_=srcS[c, :, :, :])
                for j in range(PAIRS):
                    k_ps = acc_ps.tile([128, CHW], F32, tag="acc", name="k_ps")
                    for k in range(KT):
                        nc.tensor.matmul(
                            k_ps,
                            lhsT=WK[:, k, j * 128 : (j + 1) * 128],
                            rhs=SRC[:, k, :],
                            start=(k == 0),
                            stop=(k == KT - 1),
                        )
                    nc.vector.tensor_copy(KTt[:, j, c * CHW : (c + 1) * CHW], k_ps)
                for stl in range(4):
                    st = c * 4 + stl
                    if st >= st_n:
                        break
                    v_ps = acc_ps.tile([128, CHW], F32, tag="acc", name="v_ps")
                    for k in range(KT):
                        nc.tensor.matmul(
                            v_ps[:, 0:HDL],
                            lhsT=SRC[:, k, stl * 128 : (stl + 1) * 128],
                            rhs=WV[:, k, :],
                            start=(k == 0),
                            stop=(k == KT - 1),
                        )
                    nc.vector.tensor_copy(
                        von_heads[:, st, :, 0:HD],
                        v_ps[:, 0:HDL].rearrange("p (h e) -> p h e", e=HD),
                    )

            # WO reuses WV's slot (tag) — WV is dead once the src phase ends
            WO = w_pool.tile([128, PAIRS, D], F16, tag="wv", name="WO")
            nc.sync.dma_start(out=WO, in_=woW[:, :, :])

            # ---- target projections + attention ----
            def emit_qproj_half(q_ps, QTc, TGT, j, half):
                for k in range(half * 4, half * 4 + 4):
                    nc.tensor.matmul(
                        q_ps,
                        lhsT=WQ[:, k, j * 128 : (j + 1) * 128],
                        rhs=TGT[:, k, :],
                        start=(k == 0),
                        stop=(k == KT - 1),
                    )
                if half == 1:
                    nc.vector.tensor_copy(QTc[:, j, :], q_ps)

            OTN_tiles = {}

            def emit_outproj_item(cc, ttl, dc):
                OTNp = OTN_tiles[cc]
                o_ps = acc_ps.tile([128, CHW], F32, tag="acc", name="o_ps")
                for j in range(PAIRS):
                    nc.tensor.matmul(
                        o_ps,
                        lhsT=OTNp[:, j, ttl * 128 : (ttl + 1) * 128],
                        rhs=WO[:, j, dc * CHW : (dc + 1) * CHW],
                        start=(j == 0),
                        stop=(j == PAIRS - 1),
                    )
                OSB = osb_pool.tile([128, CHW], F32, tag="osb", name="OSB")
                nc.vector.tensor_copy(OSB, o_ps)
                row0 = cc * CHW + ttl * 128
                nc.sync.dma_start(
                    out=outp[row0 : row0 + 128, dc * CHW : (dc + 1) * CHW],
                    in_=OSB,
                )

            carry = None  # (TGT, QTc) of the current chunk, prepared early
            for c in range(CH):
                if carry is None:
                    # prologue: chunk 0's Q-projection runs up front
                    TGT = stream_pool.tile(
                        [128, KT, CHW], F16, tag="stream", name="TGT"
                    )
                    nc.sync.dma_start(out=TGT, in_=tgtS[c, :, :, :])
                    QTc = qc_pool.tile([128, PAIRS, CHW], F16, name="QTc")
                    for j in range(PAIRS):
                        q_ps = acc_ps.tile([128, CHW], F32, tag="acc", name="q_ps")
                        emit_qproj_half(q_ps, QTc, TGT, j, 0)
                        emit_qproj_half(q_ps, QTc, TGT, j, 1)
                else:
                    TGT, QTc = carry

                # work items spread across this chunk's (ACT-bound) attention:
                #  - prefetch + Q-projection of chunk c+1 (1 + 8 items)
                #  - out-projection of chunk c-1 (8 items)
                items = []
                if c + 1 < CH:
                    items.append(("tgt", c + 1))
                    for j in range(PAIRS):
                        items.append(("q", j, 0))
                        items.append(("q", j, 1))
                if c > 0:
                    for ttl in range(4):
                        for dc in range(2):
                            items.append(("o", c - 1, ttl, dc))
                nxt = {}

                def pop_item():
                    if not items:
                        return
                    it = items.pop(0)
                    if it[0] == "o":
                        emit_outproj_item(it[1], it[2], it[3])
                    elif it[0] == "tgt":
                        TGTn = stream_pool.tile(
                            [128, KT, CHW], F16, tag="stream", name="TGT"
                        )
                        nc.sync.dma_start(out=TGTn, in_=tgtS[it[1], :, :, :])
                        QTn = qc_pool.tile([128, PAIRS, CHW], F16, name="QTc")
                        nxt["tgt"] = TGTn
                        nxt["qtc"] = QTn
                    else:
                        _, j, half = it
                        if half == 0:
                            nxt[("ps", j)] = acc_ps.tile(
                                [128, CHW], F32, tag="acc", name="q_ps"
                            )
                        emit_qproj_half(
                            nxt[("ps", j)], nxt["qtc"], nxt["tgt"], j, half
                        )

                OTN = on_pool.tile([128, PAIRS, CHW], F16)
                OTN_tiles[c] = OTN
                for j in range(PAIRS):
                    av = av_ps_pool.tile([128, 2 * CHW], F32, tag="av", name="av")
                    pend = None  # (PT, st) awaiting its attn@V matmuls

                    def emit_av(PT, st):
                        nc.tensor.matmul(
                            av[0:65, 0:CHW],
                            lhsT=VON[:, st, j * 130 : j * 130 + 65],
                            rhs=PT[:, 0:CHW],
                            start=(st == 0),
                            stop=(st == st_n - 1),
                        )
                        nc.tensor.matmul(
                            av[0:65, CHW : 2 * CHW],
                            lhsT=VON[:, st, j * 130 + 65 : j * 130 + 130],
                            rhs=PT[:, CHW : 2 * CHW],
                            start=(st == 0),
                            stop=(st == st_n - 1),
                        )

                    for st in range(st_n):
                        sc = sc_ps_pool.tile([128, 2 * CHW], F32, tag="sc", name="sc")
                        nc.tensor.matmul(
                            sc[:, 0:CHW],
                            lhsT=KTt[0:64, j, st * 128 : (st + 1) * 128],
                            rhs=QTc[0:64, j, :],
                            start=True,
                            stop=True,
                        )
                        nc.tensor.matmul(
                            sc[:, CHW : 2 * CHW],
                            lhsT=KTt[64:128, j, st * 128 : (st + 1) * 128],
                            rhs=QTc[64:128, j, :],
                            start=True,
                            stop=True,
                        )
                        PT = pt_pool.tile([128, 2 * CHW], F16)
                        nc.scalar.activation(PT, sc, Exp)
                        if pend is not None:
                            emit_av(*pend)
                            pop_item()
                        pend = (PT, st)
                    emit_av(*pend)

                    AVS = avs_pool.tile([128, 2 * CHW], F32, tag="avs", name="AVS")
                    nc.vector.tensor_copy(AVS[0:65, :], av[0:65, :])
                    # The custom-DVE reciprocal reads absolute partition 0 on
                    # hardware, so DMA-bounce the denominator row there first.
                    D0 = rcb_pool.tile([1, 2 * CHW], F32, tag="d0", name="D0")
                    nc.sync.dma_start(out=D0, in_=AVS[64:65, :])
                    RC = rcb_pool.tile([1, 2 * CHW], F32, tag="rc", name="RC")
                    nc.vector.reciprocal_approx_fast(RC, D0)
                    BCS = rcb_pool.tile([64, 2 * CHW], F32, tag="bcs", name="BCS")
                    nc.gpsimd.partition_broadcast(BCS, RC[0:1, :])
                    nc.vector.tensor_mul(
                        OTN[0:64, j, :], AVS[0:64, 0:CHW], BCS[:, 0:CHW]
                    )
                    STG = osb_pool.tile([128, CHW], F16, tag="stg", name="STG")
                    nc.vector.tensor_mul(
                        STG[0:64, :], AVS[0:64, CHW : 2 * CHW], BCS[:, CHW : 2 * CHW]
                    )
                    nc.sync.dma_start(out=OTN[64:128, j, :], in_=STG[0:64, :])

                # drain any leftover items before the chunk ends
                while items:
                    pop_item()
                carry = (nxt["tgt"], nxt["qtc"]) if "qtc" in nxt else None

            # final chunk's out-projection
            for ttl in range(4):
                for dc in range(2):
                    emit_outproj_item(CH - 1, ttl, dc)

    nc.finalize()
    return nc


def _get_program(st_n=None):
    global _LAST_ST_N
    if st_n is None:
        st_n = _LAST_ST_N
    if st_n not in _PROGS:
        _PROGS[st_n] = _build_program(st_n)
    _LAST_ST_N = st_n
    return _PROGS[st_n]


def kernel(src, tgt, attention_mask, Wq, Wk, Wv, Wo):
    src = np.asarray(src, dtype=np.float32)
    tgt = np.asarray(tgt, dtype=np.float32)
    mask = np.asarray(attention_mask).astype(bool)
    Wq = np.asarray(Wq, dtype=np.float32)
    Wk = np.asarray(Wk, dtype=np.float32)
    Wv = np.asarray(Wv, dtype=np.float32)
    Wo = np.asarray(Wo, dtype=np.float32)

    counts = mask.sum(axis=1)
    st_n = int(min(16, max(1, -(-int(counts.max()) // 128))))
    sp = st_n * 128
    scn = (sp + CHW - 1) // CHW
    spad = scn * CHW

    nc = _get_program(st_n)

    Wq8 = Wq * np.float32(1.0 / np.sqrt(HD))

    in_maps = []
    for core in range(N_CORES):
        b, hh = core // 2, core % 2
        rows = slice(hh * HDL, (hh + 1) * HDL)
        idx = np.nonzero(mask[b])[0]
        nb = len(idx)
        srcC = np.zeros((spad, D), dtype=np.float32)
        srcC[:nb] = src[b][idx]
        valid = np.zeros(sp, dtype=np.float16)
        valid[:nb] = 1.0
        # [T, D] -> [CH, 128p, KT, CHW]: x[c, p, k, t] = ten[c*CHW + t, k*128 + p]
        tgtSa = np.ascontiguousarray(
            tgt[b].reshape(CH, CHW, KT, 128).transpose(0, 3, 2, 1).astype(np.float16)
        )
        srcSa = np.ascontiguousarray(
            srcC.reshape(scn, CHW, KT, 128).transpose(0, 3, 2, 1).astype(np.float16)
        )
        # W[hd_local, d] -> [128p, KT, HDL]: w[p, k, h] = W[rows][h, k*128 + p]
        wqSa = np.ascontiguousarray(
            Wq8[rows].reshape(HDL, KT, 128).transpose(2, 1, 0).astype(np.float16)
        )
        wkSa = np.ascontiguousarray(
            Wk[rows].reshape(HDL, KT, 128).transpose(2, 1, 0).astype(np.float16)
        )
        wvSa = np.ascontiguousarray(
            Wv[rows].reshape(HDL, KT, 128).transpose(2, 1, 0).astype(np.float16)
        )
        # Wo[:, cols] -> [128p, PAIRS, D]: wo[p, j, d] = Wo[d, hh*HDL + j*128 + p]
        woWa = np.ascontiguousarray(
            Wo[:, rows].T.reshape(PAIRS, 128, D).transpose(1, 0, 2).astype(np.float16)
        )
        # valid01 -> [128p, st, head] (replicated across the 8 local heads)
        validSa = np.ascontiguousarray(
            np.broadcast_to(
                valid.reshape(st_n, 128).T[:, :, None], (128, st_n, HL)
            ).astype(np.float16)
        )
        in_maps.append(
            {
                "tgtS": tgtSa,
                "srcS": srcSa,
                "wqS": wqSa,
                "wkS": wkSa,
                "wvS": wvSa,
                "woW": woWa,
                "validS": validSa,
            }
        )

    global _last_in_maps
    _last_in_maps = in_maps

    res = run_bass_kernel_spmd(nc, in_maps, list(range(N_CORES)))

    out = np.empty((B, T, D), dtype=np.float32)
    for b in range(B):
        out[b] = res.results[2 * b]["outp"] + res.results[2 * b + 1]["outp"]
    return out
